# revision 20
# baseline (speedup 1.0000x reference)
"""MoE gate kernel for Trainium2 (8 NeuronCores, SPMD).

Computes, for hidden_states [4, 4096, 4096] f32 and gate_weight [8, 4096] f32:
    logits = hidden @ gate_weight.T          # [tokens, 8]
    p      = softmax(logits)                 # [tokens, 8]
    topk_w, topk_i = top_k(p, 2); topk_w /= topk_w.sum(-1, keepdims=True)

Sharding: data-parallel over tokens (B*S = 16384 -> 2048 tokens/core), gate
weight replicated.  The kernel is memory-bound and measures AT the
empirical DMA roofline: a DMA-only program streaming the same 32 MiB/core
times identically (~92-95 us/rep, ~360 GB/s effective; run-to-run drift
+-2-3 us).  All compute hides behind the x stream.

* DMA: one dma_start per 2-tile group -- [128p x (2 tiles x 4096h)] f32,
  256 contiguous 16 KB descriptors, all on the SP (sync) queue.  Measured
  on HW: SP-only beats adding ACT (+1 us/issue: it queues behind ACT's
  copies) and SWDGE/gpsimd (297 vs 364 GB/s); 16 KB vs 32/48 KB
  descriptors is a wash; 2 MiB-granularity DMAs on one queue lose
  ~7 us/rep to per-issue overhead, 16 MiB chunks serialize on buffers.
* PE, per 128x128 chunk: float32r transpose-mode matmul (1.5 cyc/row; PE
  rounds operands to ~12 mantissa bits at ingest) produces x^T in PSUM;
  4 chunks share a PSUM bank under one start/stop accumulation group.
  DVE/ACT alternate copying banks to SBUF (xtg).
* Gate: per chunk one w-stationary matmul -- lhsT = W^T chunk [128h x 8e],
  rhs = x^T [128h x 256t] streamed at 1 cyc/row (float32r needs N>=256),
  accumulating logits^T [8 x 256] over 32 chunks.  gate_order="pre": all
  of group g-1's gates + top-2 run at the START of group g's section,
  while g's DMA is in flight -- PE stays busy (no p-state downclock) and
  g-1's buffers release before the transposes start.  The last group's
  gates + the renormalization tail spill into the next rep (cross-rep
  software pipeline; the final rep drains).
* Top-2: a transpose-mode flip turns logits^T into [128t x 8e], then DVE
  max/max_index; renormalized weights reduce to w1 = 1/(1+exp(m2-m1)),
  w2 = 1-w1 (the softmax denominator cancels in the reference renorm).
* Output: weights and (bitcast) indices pack into one [P, n_tiles, 4] f32
  tensor -> a single small tail DMA per rep (two separate per-rep SWDGE
  tails measured +3.5 us/rep of DMA-pipe time on an otherwise idle pipe).

Numerics: float32r ingest rounding (HW-measured max rel 2.4e-4) flips a
handful of near-tie top-2 picks (12/32768 entries on the reference seed;
index tensor l2 rel err 1.6e-2, weights 1.1e-4 -- both inside the 2e-2
gate; near-tie swaps carry ~equal weights so weight error stays tiny).
mode="f32x" is the bit-exact fallback (~2x slower: fp32 moves through
the PE at half rate twice -- LDWEIGHTS 2-pass and 4 cyc/row streams).

Walrus's TPB instruction encodings carry a single sync-wait slot, so a
post-pass hoists surplus Tile-generated waits onto same-engine
EventSemaphore prefix instructions (semantics-preserving).
"""

import numpy as np

H = 4096            # hidden size
E = 8               # experts
P = 128             # SBUF partitions
C = H // P          # 32 h-chunks of 128
F = H // P          # 32 f32 per partition per token in strided layout
T_TILE = 128        # tokens per tile
N_CORES = 8
TOKENS_TOTAL = 4 * 4096
TOKENS_PER_CORE = TOKENS_TOTAL // N_CORES   # 2048
N_TILES = TOKENS_PER_CORE // T_TILE         # 16
CPB = 4             # transpose chunks assembled per PSUM bank
BANK_F = 512        # fp32 elems per PSUM bank per partition


def _emit_body(nc, mybir, pools, consts, x_grp, oq, n_tiles,
               carry, mode="f32x", final=True, gb=4,
               dma_engs=("sync", "scalar"), gates_per_slot=1,
               gate_order="il", tpd=None, tail_eng="scalar",
               copy_engs=("vector", "scalar")):
    natpool, xtgpool, ltpool, cpool, tpsum, gpsum, bpsum = pools
    w_nat, ident, ident_f = consts
    f32 = mybir.dt.float32
    n_groups = n_tiles // gb
    if tpd is None:
        tpd = gb
    assert gb % tpd == 0

    def emit_one_gate(prev):
        src, pgp, ptiles = prev
        c = carry["gate_c"]
        if c >= C:
            return False
        nc.tensor.matmul(
            pgp[:, 0 : gb * T_TILE], w_nat[:, c, :], src[:, c],
            start=(c == 0), stop=(c == C - 1),
        )
        carry["gate_c"] = c + 1
        return True

    def finish_group(prev):
        """Drain remaining gates, then top-2 for the previous group."""
        while emit_one_gate(prev):
            pass
        src, pgp, ptiles = prev
        lt = ltpool.tile([E, gb * T_TILE], f32, tag="lt", name="lt")
        nc.scalar.copy(lt[:], pgp[:, 0 : gb * T_TILE])
        for tg, i in enumerate(ptiles):
            pb = bpsum.tile([P, BANK_F], f32, tag="bp", name="pb")
            # transpose-mode flip [8e x 128t] -> [128t x 8e] (exact)
            nc.tensor.matmul(
                pb[:, 0:E], lt[:, tg * T_TILE : (tg + 1) * T_TILE],
                ident_f[0:E, 0:E], start=True, stop=True, is_transpose=True,
            )
            nc.vector.tensor_copy(cpool.logits[:, i, :], pb[:, 0:E])
            nc.vector.max(out=cpool.sorted_w[:, i], in_=cpool.logits[:, i, :])
            nc.vector.max_index(
                out=cpool.idx_w[:, i], in_max=cpool.sorted_w[:, i],
                in_values=cpool.logits[:, i, :],
            )

    # DMA granularity: tpd tiles per dma_start ([P, tpd, H] natural tiles,
    # tpd*128 descriptors of 16 KB each).  tpd < gb lets the first tile's
    # transposes start before the whole gate group has landed.
    engs = [getattr(nc, e) for e in dma_engs]
    n_dma = [0]

    def load_tiles(g):
        """Issue the DMAs for group g; returns per-tile [P, H] views."""
        views = []
        for d in range(gb // tpd):
            xg = natpool.tile([P, tpd, H], cpool.dt_x, tag="xt", name="xt")
            engs[n_dma[0] % len(engs)].dma_start(
                xg[:], x_grp[g * (gb // tpd) + d]
            )
            n_dma[0] += 1
            views.extend(xg[:, j] for j in range(tpd))
        return views

    for g in range(n_groups):
        tiles = [g * gb + k for k in range(gb)]
        gp = gpsum.tile([E, BANK_F], f32, tag="gp", name="gp")
        prev = carry["prev"]
        xts = load_tiles(g)
        xtg = xtgpool.tile([P, C, gb, T_TILE], cpool.dt_x, tag="xtg",
                           name="xtg")
        cur = (xtg, gp, tiles)
        if gate_order == "pre" and prev is not None:
            # All of prev's gates + top-2 run while this group's DMA is in
            # flight: PE stays busy (no p-state drop) and prev's xtg is
            # released before the transposes start.
            finish_group(prev)
        asm = 0
        for tg, xt in enumerate(xts):
            for cb in range(C // CPB):
                pst = tpsum.tile([P, CPB, T_TILE], cpool.dt_x, tag="pst",
                                 name="pst")
                for k in range(CPB):
                    c = cb * CPB + k
                    nc.tensor.matmul(
                        pst[:, k], xt[:, c * P : (c + 1) * P], ident[:],
                        start=(k == 0), stop=(k == CPB - 1),
                        is_transpose=True,
                    )
                dst = xtg[:, cb * CPB : (cb + 1) * CPB, tg, :]
                if copy_engs[asm % len(copy_engs)] == "vector":
                    nc.vector.tensor_copy(dst, pst[:])
                else:
                    nc.scalar.copy(dst, pst[:])
                asm += 1
                if gate_order != "pre" and prev is not None:
                    for _ in range(gates_per_slot):
                        emit_one_gate(prev)
        if gate_order != "pre" and prev is not None:
            finish_group(prev)
        carry["prev"] = cur
        carry["gate_c"] = 0
        if g == 0 and carry.get("pending_tail"):
            _emit_rep_tail(nc, mybir, cpool, oq, n_tiles, tail_eng)
            carry["pending_tail"] = False

    if final:
        finish_group(carry["prev"])
        carry["prev"] = None
        _emit_rep_tail(nc, mybir, cpool, oq, n_tiles, tail_eng)
    else:
        carry["pending_tail"] = True


def _emit_rep_tail(nc, mybir, cpool, oq, n_tiles, tail_eng="scalar"):
    f32 = mybir.dt.float32
    sorted_w, idx_w = cpool.sorted_w, cpool.idx_w
    # Batched renormalization over all tiles: w1 = 1/(1+e^(m2-m1)),
    # w2 = e^(m2-m1)/(1+e^(m2-m1)).
    m1 = sorted_w[:, :, 0]
    m2 = sorted_w[:, :, 1]
    d = cpool.tile([P, n_tiles], f32, tag="d", name="d")
    nc.vector.tensor_sub(d[:], m2, m1)
    t = cpool.tile([P, n_tiles], f32, tag="t", name="t")
    nc.scalar.activation(t[:], d[:], mybir.ActivationFunctionType.Exp)
    denom = cpool.tile([P, n_tiles], f32, tag="denom", name="denom")
    nc.vector.tensor_scalar_add(denom[:], t[:], 1.0)
    r = cpool.tile([P, n_tiles], f32, tag="r", name="r")
    nc.vector.reciprocal(r[:], denom[:])

    # single packed output [w1, w2, i1, i2] -> one small tail DMA per rep
    # (two per-rep SWDGE tails measured +3.5 us/rep of DMA-pipe time)
    wout = cpool.tile([P, n_tiles, 4], f32, tag="wout", name="wout")
    nc.vector.tensor_copy(wout[:, :, 0], r[:])
    nc.vector.tensor_mul(wout[:, :, 1], t[:], r[:])
    nc.vector.tensor_copy(
        wout[:, :, 2:4].bitcast(mybir.dt.uint32), idx_w[:, :, 0:2]
    )
    getattr(nc, tail_eng).dma_start(oq[:], wout[:])


def _legalize_sync_waits(nc, mybir):
    """Split surplus sync waits onto EventSemaphore prefix instructions.

    Walrus's TPB instruction structs have a single `events` wait slot and
    reject instructions with more sync waits.  The same engine sequencer
    executes an EventSemaphore (CTRL_ES) wait-only instruction in program
    order, so hoisting all but one wait onto ES prefixes is
    semantics-preserving.
    """
    limit = 1
    n = 0
    for bb in nc.main_func.blocks:
        out, changed = [], False
        for ins in bb.instructions:
            si = ins.sync_info
            if si is not None and len(si.on_wait) > limit:
                waits = list(si.on_wait)
                for w in waits[:-limit]:
                    es = mybir.InstEventSemaphore(
                        name=f"ESleg-{n}", engine=ins.engine, ins=[], outs=[],
                        sync_info=mybir.SyncInfo(on_wait=[w], on_update=[]),
                    )
                    out.append(es)
                    n += 1
                ins.sync_info = mybir.SyncInfo(
                    on_wait=waits[-limit:], on_update=list(si.on_update)
                )
                changed = True
            out.append(ins)
        if changed:
            bb.instructions = out
    return n


def build_program(tokens_per_core: int = TOKENS_PER_CORE, reps: int = 1,
                  legalize: bool = True, mode: str = "f32x",
                  nat_bufs: int = 3, tp_bufs: int = 3,
                  xtg_bufs: int = 2, gb: int = 2,
                  dma_engs=("sync", "scalar"), gates_per_slot: int = 1,
                  gate_order: str = "il", tpd: int = None,
                  tail_eng: str = "scalar",
                  copy_engs=("vector", "scalar")):
    import concourse.bass as bass
    import concourse.mybir as mybir
    from concourse.masks import make_identity
    from concourse.tile import TileContext

    f32 = mybir.dt.float32
    f32r = mybir.dt.float32r
    dt_x = f32r if mode == "f32r" else f32
    n_tiles = tokens_per_core // T_TILE
    nc = bass.Bass("TRN2", debug=False)
    x = nc.declare_dram_parameter(
        "x", [tokens_per_core, H], dt_x, isOutput=False
    )
    w = nc.declare_dram_parameter("w", [E, H], dt_x, isOutput=False)
    oq = nc.declare_dram_parameter(
        "oq", [P, n_tiles, 4], f32, isOutput=True
    )
    if tpd is None:
        tpd = gb
    # DMA-unit layout: x_grp[n][p, tg, h] = x[n*tpd*128 + tg*128 + p, h]
    x_grp = x[:].rearrange("(n tg p) h -> n p tg h", tg=tpd, p=P)

    with TileContext(nc) as tc:
        with tc.tile_pool(name="cpool", bufs=1) as cpool:
            cpool.dt_x = dt_x
            ident_f = cpool.tile([P, P], f32, name="ident_f")
            make_identity(nc, ident_f[:])
            if mode == "f32r":
                ident = cpool.tile([P, P], f32r, name="ident")
                nc.sync.dma_start(ident[:], ident_f[:].bitcast(f32r))
            else:
                ident = ident_f
            w_nat = cpool.tile([P, C, E], dt_x, name="w_nat")
            cpool.logits = cpool.tile([P, n_tiles, E], f32, name="logits")
            cpool.sorted_w = cpool.tile([P, n_tiles, E], f32, name="sortw")
            cpool.idx_w = cpool.tile(
                [P, n_tiles, E], mybir.dt.uint32, name="idxw"
            )

            # one-time W^T build (natural-chunk layout), pools released after
            with (
                tc.tile_pool(name="wbuild", bufs=1) as wb,
                tc.tile_pool(name="wbps", bufs=2, space="PSUM") as wbps,
            ):
                w_rows = wb.tile([E, H], dt_x, name="w_rows")
                nc.sync.dma_start(w_rows[:], w[:])
                for c in range(C):
                    psw = wbps.tile([P, BANK_F], dt_x, tag="bp", name="psw")
                    nc.tensor.matmul(
                        psw[:, 0:E], w_rows[:, c * P : (c + 1) * P],
                        ident[0:E, 0:E], start=True, stop=True,
                        is_transpose=True,
                    )
                    nc.vector.tensor_copy(w_nat[:, c, :], psw[:, 0:E])

            with (
                tc.tile_pool(name="natpool", bufs=nat_bufs) as natpool,
                tc.tile_pool(name="xtgpool", bufs=xtg_bufs) as xtgpool,
                tc.tile_pool(name="ltpool", bufs=2) as ltpool,
                tc.tile_pool(name="tpsum", bufs=tp_bufs, space="PSUM") as tpsum,
                tc.tile_pool(name="gpsum", bufs=2, space="PSUM") as gpsum,
                tc.tile_pool(name="bpsum", bufs=2, space="PSUM") as bpsum,
            ):
                pools = (natpool, xtgpool, ltpool, cpool,
                         tpsum, gpsum, bpsum)
                carry = {"prev": None, "gate_c": 0}
                for _rep in range(reps):
                    _emit_body(
                        nc, mybir, pools, (w_nat, ident, ident_f),
                        x_grp, oq, n_tiles, carry,
                        mode=mode, final=(_rep == reps - 1),
                        gb=gb, dma_engs=dma_engs,
                        gates_per_slot=gates_per_slot,
                        gate_order=gate_order, tpd=tpd, tail_eng=tail_eng,
                        copy_engs=copy_engs,
                    )
    if legalize:
        _legalize_sync_waits(nc, mybir)
    return nc


def shard_inputs(hidden_states, gate_weight):
    hs = np.ascontiguousarray(np.asarray(hidden_states, dtype=np.float32)).reshape(
        TOKENS_TOTAL, H
    )
    gw = np.ascontiguousarray(np.asarray(gate_weight, dtype=np.float32))
    return [
        {"x": hs[c * TOKENS_PER_CORE : (c + 1) * TOKENS_PER_CORE], "w": gw}
        for c in range(N_CORES)
    ]


def assemble(results):
    ws, idxs = [], []
    for c in range(N_CORES):
        oq = np.ascontiguousarray(
            np.asarray(results[c]["oq"]).reshape(P, N_TILES, 4)
        )
        wq = oq[:, :, 0:2]
        iq = np.ascontiguousarray(oq[:, :, 2:4]).view(np.uint32)
        # token (core-local) = tile*128 + p
        ws.append(np.transpose(wq, (1, 0, 2)).reshape(TOKENS_PER_CORE, 2))
        idxs.append(np.transpose(iq, (1, 0, 2)).reshape(TOKENS_PER_CORE, 2))
    w_full = np.concatenate(ws, 0).reshape(4, 4096, 2).astype(np.float32)
    i_full = np.concatenate(idxs, 0).reshape(4, 4096, 2).astype(np.int32)
    return w_full, i_full


BEST_CONFIG = {"mode": "f32r", "tp_bufs": 4, "gb": 2,
               "dma_engs": ("sync",), "gates_per_slot": 4,
               "gate_order": "il"}


def kernel(hidden_states, gate_weight):
    from concourse.bass_utils import run_bass_kernel_spmd

    nc = build_program(**BEST_CONFIG)
    in_maps = shard_inputs(hidden_states, gate_weight)
    br = run_bass_kernel_spmd(nc, in_maps, list(range(N_CORES)), trace=False)
    return assemble(br.results)

